# revision 19
# baseline (speedup 1.0000x reference)
"""GraphSAGE (3-layer) Trainium2 Bass kernel, 8-core SPMD. v3

Strategy (graph/data parallel):
  - Nodes padded to 50176 = 8*6272; core c owns dst nodes [c*6272, (c+1)*6272),
    49 dst tiles of 128 nodes per core.
  - Mean-aggregation per dst tile as PE matmuls: psum += oh_k.T @ msg_k over
    chunks k of 128 gathered rows, msg = dma_gather(h_table[row_src]).
  - GPSIMD descriptor generation (~7.8ns/idx, the wall) is minimized:
      * per-(tile,group) gather stream lengths are the exact cross-core max,
        rounded to 16 (not 128): no ceil-to-128 quantization padding;
      * edges with the same (dst tile, src) are pair-deduplicated: one gathered
        row serves two destination slots via a SECOND one-hot pass whose
        matmuls reuse the same msg chunks (multi-dst rows are packed first).
  - One-hot built in ONE DVE op per tile (+1 small op for the second pass):
    oh[p, k, s] = is_equal(iota[s], dstloc[p, k]) with stride-0 broadcast APs.
    Pad slots carry dstloc=128 -> zero one-hot row (gathered slot garbage is
    zeroed once at start and multiplied by zero afterwards).
  - Activations flow FEATURE-major (stageT [f, tile, node]): linear layers run
    directly (lhsT=W[in_f, out_f], rhs=stageT), ReLU+bias on ACT writes the
    next stage in place. 2 PE transposes per tile (hn, table-row write).
  - The h table is split (A = tiles 0..23 per core, B = tiles 24..48).
    AllGather A is issued mid-layer so it overlaps the back half of the tile
    loop; next layer's lo-gathers depend only on it, hi-gathers on AllGather B.
"""

import sys

if "/opt/trn_rl_repo" not in sys.path:
    sys.path.insert(0, "/opt/trn_rl_repo")

from contextlib import ExitStack

import numpy as np
import ml_dtypes

N_NODES = 50000
F = 128
OUT_F = 64
NCORES = 8
NLOC = 6272          # nodes per core
NTILES = 49          # 6272 / 128
NPAD = NCORES * NLOC  # 50176
P = 128
AT = 32              # tiles per core in table A (NA = 32768 = int16 ceiling)
BT = NTILES - AT     # 17 tiles in table B (small -> fast end-of-layer collective)
RA = AT * P          # 4096 rows per core in A
RB = BT * P          # 2176 rows per core in B
NA = NCORES * RA     # 32768
NB = NCORES * RB     # 17408
NGRP = NCORES * NTILES * 2  # (tile, lo/hi) buckets

_prog_cache = {}


def _wrap_idx_flat(a):
    """[n] idx stream (n % 16 == 0) -> wrapped [128, n/16] int16."""
    n = a.shape[0]
    w = a.reshape(n // 16, 16).T            # [16, n/16]
    w = np.tile(w, (8, 1))                  # [128, n/16]
    return np.ascontiguousarray(w.astype(np.int16))


def _preprocess(src, dst):
    """Bucket edges by (core,tile,lo/hi), pair-dedup same-src edges, build
    variable-length gather streams (exact cross-core max, x16)."""
    bf = ml_dtypes.bfloat16
    src = src.astype(np.int64)
    dst = dst.astype(np.int64)
    E = src.shape[0]

    gtile = dst // P
    dstloc = dst % P
    c_src = src // NLOC
    r_src = src % NLOC
    lo = r_src < RA
    tabidx = np.where(lo, c_src * RA + r_src, c_src * RB + (r_src - RA))
    bucket = gtile * 2 + (~lo).astype(np.int64)     # 0..783

    # occurrence index within (bucket, tabidx)
    ord1 = np.lexsort((tabidx, bucket))
    b_s = bucket[ord1]
    s_s = tabidx[ord1]
    d_s = dstloc[ord1]
    new = np.ones(E, bool)
    new[1:] = (b_s[1:] != b_s[:-1]) | (s_s[1:] != s_s[:-1])
    runid = np.cumsum(new) - 1
    runstart = np.flatnonzero(new)
    occ = np.arange(E) - runstart[runid]

    # rows: one per (bucket, src, occ//2). slot2 (occ%2==1) rides as 2nd dst.
    is_row = (occ % 2) == 0
    nxt_same = np.zeros(E, bool)
    nxt_same[:-1] = ~new[1:]
    has2_stream = is_row & nxt_same
    d2_stream = np.empty(E, np.int64)
    d2_stream[:-1] = d_s[1:]
    d2_stream[-1] = P
    rows_b = b_s[is_row]
    rows_s = s_s[is_row]
    rows_d1 = d_s[is_row]
    rows_h2 = has2_stream[is_row]
    rows_d2 = np.where(rows_h2, d2_stream[is_row], P)

    # group rows per bucket, multi-dst rows first
    ord2 = np.lexsort((~rows_h2, rows_b))
    rb = rows_b[ord2]
    rs = rows_s[ord2]
    rd1 = rows_d1[ord2]
    rd2 = rows_d2[ord2]
    rcnt = np.bincount(rb, minlength=NGRP)                 # rows per bucket
    r2cnt = np.bincount(rb[rows_h2[ord2]], minlength=NGRP)  # 2nd-dst rows
    rstart = np.zeros(NGRP + 1, np.int64)
    np.cumsum(rcnt, out=rstart[1:])
    rpos = np.arange(len(rb)) - rstart[rb]

    # per-(tile,grp) stream length: exact max over cores, x16, >= 16
    rc = rcnt.reshape(NCORES, NTILES, 2)
    r2c = r2cnt.reshape(NCORES, NTILES, 2)
    lens = rc.max(axis=0)                     # [NTILES, 2]
    lens = np.maximum((lens + 15) // 16 * 16, 16)
    nch = -(-lens // P)                       # chunks per (tile, grp)
    nx = -(-r2c.max(axis=0) // P)             # 2nd-pass chunks per (tile, grp)

    lenlo = tuple(int(x) for x in lens[:, 0])
    lenhi = tuple(int(x) for x in lens[:, 1])
    nxlo = tuple(int(x) for x in nx[:, 0])
    nxhi = tuple(int(x) for x in nx[:, 1])

    # slot arrays, concatenated variable-width per (tile, grp)
    off_idx = np.zeros((NTILES, 2), np.int64)      # idx-stream offsets
    off_ch = np.zeros((NTILES, 2), np.int64)       # chunk offsets (main)
    off_x = np.zeros((NTILES, 2), np.int64)        # chunk offsets (pass 2)
    acc_i = acc_c = acc_x = 0
    for t in range(NTILES):
        for g in range(2):
            off_idx[t, g] = acc_i
            off_ch[t, g] = acc_c
            off_x[t, g] = acc_x
            acc_i += lens[t, g]
            acc_c += nch[t, g]
            acc_x += nx[t, g]
    TOTI, TOTCH, TOTX = acc_i, acc_c, max(acc_x, 1)

    idx_slot = np.zeros((NCORES, TOTI), np.int64)
    oh_slot = np.full((NCORES, TOTCH * P), P, np.int64)
    oh2_slot = np.full((NCORES, TOTX * P), P, np.int64)

    core_of = rb // (NTILES * 2)
    t_of = (rb // 2) % NTILES
    g_of = rb % 2
    col_i = off_idx[t_of, g_of] + rpos
    idx_slot[core_of, col_i] = rs
    col_o = off_ch[t_of, g_of] * P + rpos
    oh_slot[core_of, col_o] = rd1
    m2 = (rd2 != P) & (rpos < nx[t_of, g_of] * P)
    col_x = off_x[t_of, g_of] * P + rpos
    oh2_slot[core_of[m2], col_x[m2]] = rd2[m2]

    deg = np.bincount(dst, minlength=NPAD).astype(np.float32)
    inv_deg = 1.0 / np.maximum(deg, 1.0)

    per_core = []
    for c in range(NCORES):
        idxs = _wrap_idx_flat(idx_slot[c])                     # [128, TOTI/16]
        dstlocb = np.ascontiguousarray(
            oh_slot[c].reshape(TOTCH, P).T).astype(bf)         # [128, TOTCH]
        dstlocb2 = np.ascontiguousarray(
            oh2_slot[c].reshape(TOTX, P).T).astype(bf)         # [128, TOTX]
        invd = inv_deg[c * NLOC:(c + 1) * NLOC].reshape(NTILES, P).T.copy()
        per_core.append(dict(idxs=idxs, dstlocb=dstlocb, dstlocb2=dstlocb2,
                             invdeg=invd))
    shape_key = (lenlo, lenhi, nxlo, nxhi)
    return per_core, shape_key


def _build_program(shape_key):
    import concourse.bacc as bacc
    import concourse.mybir as mybir
    import concourse.tile as tile

    lenlo, lenhi, nxlo, nxhi = shape_key
    nch_lo = [-(-v // P) for v in lenlo]
    nch_hi = [-(-v // P) for v in lenhi]
    NCLOMAX = max(nch_lo)
    NCHIMAX = max(nch_hi)
    NXMAX = max(a + b for a, b in zip(nxlo, nxhi))
    KLAG = 7    # lo-gathers run this many tiles ahead of hi-gathers
    HLAG = 2    # hi-gather to compute lag
    TOTI = sum(lenlo) + sum(lenhi)
    TOTCH = sum(nch_lo) + sum(nch_hi)
    TOTX = max(sum(nxlo) + sum(nxhi), 1)
    # offsets in emission order (t, lo), (t, hi)
    off_i, off_c, off_x = {}, {}, {}
    ai = ac = ax = 0
    for t in range(NTILES):
        for g, (ln, nc_, nx_) in enumerate((
                (lenlo[t], nch_lo[t], nxlo[t]), (lenhi[t], nch_hi[t], nxhi[t]))):
            off_i[t, g] = ai
            off_c[t, g] = ac
            off_x[t, g] = ax
            ai += ln
            ac += nc_
            ax += nx_

    dt = mybir.dt
    nc = bacc.Bacc("TRN2", target_bir_lowering=False, debug=False,
                   num_devices=NCORES, dynamic_dma_scratch_size=49152,
                   num_swdge_queues=4)

    htabA0 = nc.dram_tensor("htabA0", [NA, F], dt.bfloat16, kind="ExternalInput")
    htabB0 = nc.dram_tensor("htabB0", [NB, F], dt.bfloat16, kind="ExternalInput")
    hselfT0 = nc.dram_tensor("hselfT0", [F, NTILES, P], dt.bfloat16, kind="ExternalInput")
    idxs_d = nc.dram_tensor("idxs", [P, TOTI // 16], dt.int16, kind="ExternalInput")
    dstlocb = nc.dram_tensor("dstlocb", [P, TOTCH], dt.bfloat16, kind="ExternalInput")
    dstlocb2 = nc.dram_tensor("dstlocb2", [P, TOTX], dt.bfloat16, kind="ExternalInput")
    iotam = nc.dram_tensor("iotam", [P, P], dt.bfloat16, kind="ExternalInput")
    invdeg = nc.dram_tensor("invdeg", [P, NTILES], dt.float32, kind="ExternalInput")
    ident = nc.dram_tensor("ident", [P, P], dt.bfloat16, kind="ExternalInput")
    ws = [nc.dram_tensor(f"ws{l}", [F, F if l < 2 else OUT_F], dt.bfloat16,
                         kind="ExternalInput") for l in range(3)]
    wn = [nc.dram_tensor(f"wn{l}", [F, F if l < 2 else OUT_F], dt.bfloat16,
                         kind="ExternalInput") for l in range(3)]
    bs = [nc.dram_tensor(f"b{l}", [F if l < 2 else OUT_F, 1], dt.float32,
                         kind="ExternalInput") for l in range(3)]
    outd = nc.dram_tensor("outT", [OUT_F, NTILES, P], dt.float32, kind="ExternalOutput")

    tabsA = [htabA0,
             nc.dram_tensor("htabA1", [NA, F], dt.bfloat16, addr_space="Shared"),
             nc.dram_tensor("htabA2", [NA, F], dt.bfloat16, addr_space="Shared")]
    tabsB = [htabB0,
             nc.dram_tensor("htabB1", [NB, F], dt.bfloat16, addr_space="Shared"),
             nc.dram_tensor("htabB2", [NB, F], dt.bfloat16, addr_space="Shared")]
    blkA = [nc.dram_tensor(f"blkA{l}", [RA, F], dt.bfloat16) for l in range(2)]
    blkB = [nc.dram_tensor(f"blkB{l}", [RB, F], dt.bfloat16) for l in range(2)]

    with tile.TileContext(nc) as tc, ExitStack() as ctx:
        const = ctx.enter_context(tc.tile_pool(name="const", bufs=1))
        stpool = ctx.enter_context(tc.tile_pool(name="stage", bufs=1))
        NLOBUF = KLAG + HLAG + 2
        NHIBUF = HLAG + 2
        msgpl = ctx.enter_context(tc.tile_pool(name="msglo", bufs=NLOBUF))
        msgph = ctx.enter_context(tc.tile_pool(name="msghi", bufs=NHIBUF))
        ohp = ctx.enter_context(tc.tile_pool(name="oh", bufs=2))
        sbw = ctx.enter_context(tc.tile_pool(name="work", bufs=4))
        psA = ctx.enter_context(tc.tile_pool(name="psA", bufs=2, space="PSUM"))
        psT = ctx.enter_context(tc.tile_pool(name="psT", bufs=2, space="PSUM"))
        psO = ctx.enter_context(tc.tile_pool(name="psO", bufs=2, space="PSUM"))

        def load(t, d):
            nc.sync.dma_start(t[:], d[:])
            return t

        idxs_sb = load(const.tile([P, TOTI // 16], dt.int16, name="idxs_sb"), idxs_d)
        dstlocb_sb = load(const.tile([P, TOTCH], dt.bfloat16, name="dstlocb_sb"), dstlocb)
        dstlocb2_sb = load(const.tile([P, TOTX], dt.bfloat16, name="dstlocb2_sb"), dstlocb2)
        iotam_sb = load(const.tile([P, P], dt.bfloat16, name="iotam_sb"), iotam)
        invdeg_sb = load(const.tile([P, NTILES], dt.float32, name="invdeg_sb"), invdeg)
        ident_sb = load(const.tile([P, P], dt.bfloat16, name="ident_sb"), ident)
        ws_sb = [load(const.tile([F, F if l < 2 else OUT_F], dt.bfloat16, name=f"ws_sb{l}"), ws[l])
                 for l in range(3)]
        wn_sb = [load(const.tile([F, F if l < 2 else OUT_F], dt.bfloat16, name=f"wn_sb{l}"), wn[l])
                 for l in range(3)]
        bs_sb = [load(const.tile([F if l < 2 else OUT_F, 1], dt.float32, name=f"bs_sb{l}"), bs[l])
                 for l in range(3)]

        stageA = load(stpool.tile([F, NTILES, P], dt.bfloat16, name="stageA", tag="stA"), hselfT0)
        stageB = stpool.tile([F, NTILES, P], dt.bfloat16, tag="stB")
        outstage = stpool.tile([OUT_F, NTILES, P], dt.float32, tag="stO")

        # prime msg buffers: pad slots in partially-gathered chunks are never
        # written and must stay finite (0 * 0 = 0 in the matmul).
        for _ in range(NLOBUF):
            m = msgpl.tile([P, NCLOMAX, F], dt.bfloat16, tag="msglo")
            nc.vector.memset(m[:], 0.0)
        for _ in range(NHIBUF):
            m = msgph.tile([P, NCHIMAX, F], dt.bfloat16, tag="msghi")
            nc.vector.memset(m[:], 0.0)

        OHMAX = max(nch_lo[t] + nch_hi[t] + nxlo[t] + nxhi[t]
                    for t in range(NTILES))
        COLL_A_POS = AT + KLAG + HLAG  # blkA rows complete at position AT-1+K+H

        stage_prev, stage_next = stageA, stageB
        qn = 0
        for l in range(3):
            tabA, tabB = tabsA[l], tabsB[l]
            OUTL = F if l < 2 else OUT_F
            lo_tiles, hi_tiles = {}, {}
            for j in range(NTILES + KLAG + HLAG):
                if j < NTILES:
                    mlo = msgpl.tile([P, NCLOMAX, F], dt.bfloat16, tag="msglo")
                    nc.gpsimd.dma_gather(
                        mlo[:, 0:nch_lo[j], :], tabA[:, :],
                        idxs_sb[:, off_i[j, 0] // 16:(off_i[j, 0] + lenlo[j]) // 16],
                        num_idxs=lenlo[j], num_idxs_reg=lenlo[j], elem_size=F,
                        single_packet=False, queue_num=qn % 4)
                    qn += 1
                    lo_tiles[j] = mlo
                if KLAG <= j < NTILES + KLAG:
                    t = j - KLAG
                    mhi = msgph.tile([P, NCHIMAX, F], dt.bfloat16, tag="msghi")
                    nc.gpsimd.dma_gather(
                        mhi[:, 0:nch_hi[t], :], tabB[:, :],
                        idxs_sb[:, off_i[t, 1] // 16:(off_i[t, 1] + lenhi[t]) // 16],
                        num_idxs=lenhi[t], num_idxs_reg=lenhi[t], elem_size=F,
                        single_packet=False, queue_num=qn % 4)
                    qn += 1
                    hi_tiles[t] = mhi

                if l < 2 and j == COLL_A_POS:
                    nc.gpsimd.collective_compute(
                        "AllGather", mybir.AluOpType.bypass,
                        replica_groups=[list(range(NCORES))],
                        ins=[blkA[l][:]], outs=[tabsA[l + 1][:]])

                if j < KLAG + HLAG:
                    continue
                t = j - KLAG - HLAG
                nlo, nhi = nch_lo[t], nch_hi[t]
                ncht = nlo + nhi
                mlo, mhi = lo_tiles.pop(t), hi_tiles.pop(t)

                oh = ohp.tile([P, OHMAX, P], dt.bfloat16, tag="oh")
                nc.vector.tensor_tensor(
                    oh[:, 0:ncht, :],
                    iotam_sb[:].unsqueeze(1).to_broadcast([P, ncht, P]),
                    dstlocb_sb[:, off_c[t, 0]:off_c[t, 0] + ncht]
                    .unsqueeze(2).to_broadcast([P, ncht, P]),
                    mybir.AluOpType.is_equal)
                nxt = nxlo[t] + nxhi[t]
                if nxt:
                    nc.vector.tensor_tensor(
                        oh[:, ncht:ncht + nxt, :],
                        iotam_sb[:].unsqueeze(1).to_broadcast([P, nxt, P]),
                        dstlocb2_sb[:, off_x[t, 0]:off_x[t, 0] + nxt]
                        .unsqueeze(2).to_broadcast([P, nxt, P]),
                        mybir.AluOpType.is_equal)

                # (one-hot chunk, msg tile+chunk) pairs: main + 2nd-dst pass
                pairs = [(k, mlo, k) for k in range(nlo)]
                pairs += [(nlo + k, mhi, k) for k in range(nhi)]
                pairs += [(ncht + k, mlo, k) for k in range(nxlo[t])]
                pairs += [(ncht + nxlo[t] + k, mhi, k) for k in range(nxhi[t])]
                agg = psA.tile([P, F], dt.float32, tag="agg")
                for i, (ko, mt, km) in enumerate(pairs):
                    nc.tensor.matmul(agg[:], oh[:, ko, :], mt[:, km, :],
                                     start=(i == 0), stop=(i == len(pairs) - 1))
                hn = sbw.tile([P, F], dt.bfloat16, tag="hn")
                nc.vector.tensor_scalar_mul(hn[:], agg[:], invdeg_sb[:, t:t + 1])

                hnT_ps = psT.tile([F, P], dt.bfloat16, tag="hnT")
                nc.tensor.transpose(hnT_ps[:], hn[:], ident_sb[:])
                hnT = sbw.tile([F, P], dt.bfloat16, tag="hnTs")
                nc.vector.tensor_copy(hnT[:], hnT_ps[:])

                outp = psO.tile([OUTL, P], dt.float32, tag="outp")
                nc.tensor.matmul(outp[:], ws_sb[l][:], stage_prev[:, t, :],
                                 start=True, stop=False)
                nc.tensor.matmul(outp[:], wn_sb[l][:], hnT[:],
                                 start=False, stop=True)

                if l < 2:
                    nc.scalar.activation(stage_next[:, t, :], outp[:],
                                         mybir.ActivationFunctionType.Relu,
                                         bias=bs_sb[l][:], scale=1.0)
                    oT_ps = psT.tile([P, F], dt.bfloat16, tag="oT")
                    nc.tensor.transpose(oT_ps[:], stage_next[:, t, :], ident_sb[:])
                    rowm = sbw.tile([P, F], dt.bfloat16, tag="rowm")
                    nc.vector.tensor_copy(rowm[:], oT_ps[:])
                    if t < AT:
                        nc.sync.dma_start(blkA[l][t * P:(t + 1) * P, :], rowm[:])
                    else:
                        nc.sync.dma_start(
                            blkB[l][(t - AT) * P:(t - AT + 1) * P, :], rowm[:])
                else:
                    nc.vector.tensor_scalar_add(outstage[:, t, :], outp[:],
                                                bs_sb[2][:])

            if l < 2:
                nc.gpsimd.collective_compute(
                    "AllGather", mybir.AluOpType.bypass,
                    replica_groups=[list(range(NCORES))],
                    ins=[blkB[l][:]], outs=[tabsB[l + 1][:]])
                stage_prev, stage_next = stage_next, stage_prev

        nc.sync.dma_start(outd[:], outstage[:])

    nc.compile()
    return nc


def kernel(features, src, dst, W0, b0, W1, b1, W2, b2):
    features = np.asarray(features, np.float32)
    src = np.asarray(src)
    dst = np.asarray(dst)

    per_core, shape_key = _preprocess(src, dst)

    if shape_key not in _prog_cache:
        _prog_cache[shape_key] = _build_program(shape_key)
    nc = _prog_cache[shape_key]

    bf = ml_dtypes.bfloat16
    feat_pad = np.zeros((NPAD, F), np.float32)
    feat_pad[:N_NODES] = features
    fp = feat_pad.reshape(NCORES, NLOC, F)
    htabA0 = np.ascontiguousarray(fp[:, :RA].reshape(NA, F)).astype(bf)
    htabB0 = np.ascontiguousarray(fp[:, RA:].reshape(NB, F)).astype(bf)
    ident = np.eye(P, dtype=bf)
    iotam = np.tile(np.arange(P, dtype=np.float32), (P, 1)).astype(bf)
    Wl = [np.asarray(w, np.float32) for w in (W0, W1, W2)]
    bl = [np.asarray(b, np.float32).reshape(-1, 1) for b in (b0, b1, b2)]

    common = dict(htabA0=htabA0, htabB0=htabB0, iotam=iotam, ident=ident)
    for l in range(3):
        common[f"ws{l}"] = Wl[l][:F].astype(bf)
        common[f"wn{l}"] = Wl[l][F:].astype(bf)
        common[f"b{l}"] = bl[l]

    in_maps = []
    for c in range(NCORES):
        m = dict(common)
        m.update(per_core[c])
        hs = feat_pad[c * NLOC:(c + 1) * NLOC].reshape(NTILES, P, F)
        m["hselfT0"] = np.ascontiguousarray(hs.transpose(2, 0, 1)).astype(bf)
        in_maps.append(m)

    from concourse.bass_utils import run_bass_kernel_spmd
    res = run_bass_kernel_spmd(nc, in_maps, core_ids=list(range(NCORES)))
    global last_result
    last_result = res
    out = np.empty((NPAD, OUT_F), np.float32)
    for c in range(NCORES):
        oT = res.results[c]["outT"]  # [OUT_F, NTILES, P]
        out[c * NLOC:(c + 1) * NLOC] = np.asarray(oT).transpose(1, 2, 0).reshape(NLOC, OUT_F)
    return np.ascontiguousarray(out[:N_NODES]).astype(np.float32)


last_result = None


# revision 20
# speedup vs baseline: 1.1064x; 1.1064x over previous
"""GraphSAGE (3-layer) Trainium2 Bass kernel, 8-core SPMD. v3

Strategy (graph/data parallel):
  - Nodes padded to 50176 = 8*6272; core c owns dst nodes [c*6272, (c+1)*6272),
    49 dst tiles of 128 nodes per core.
  - Mean-aggregation per dst tile as PE matmuls: psum += oh_k.T @ msg_k over
    chunks k of 128 gathered rows, msg = dma_gather(h_table[row_src]).
  - GPSIMD descriptor generation (~7.8ns/idx, the wall) is minimized:
      * per-(tile,group) gather stream lengths are the exact cross-core max,
        rounded to 16 (not 128): no ceil-to-128 quantization padding;
      * edges with the same (dst tile, src) are pair-deduplicated: one gathered
        row serves two destination slots via a SECOND one-hot pass whose
        matmuls reuse the same msg chunks (multi-dst rows are packed first).
  - One-hot built in ONE DVE op per tile (+1 small op for the second pass):
    oh[p, k, s] = is_equal(iota[s], dstloc[p, k]) with stride-0 broadcast APs.
    Pad slots carry dstloc=128 -> zero one-hot row (gathered slot garbage is
    zeroed once at start and multiplied by zero afterwards).
  - Activations flow FEATURE-major (stageT [f, tile, node]): linear layers run
    directly (lhsT=W[in_f, out_f], rhs=stageT), ReLU+bias on ACT writes the
    next stage in place. 2 PE transposes per tile (hn, table-row write).
  - The h table is split (A = tiles 0..23 per core, B = tiles 24..48).
    AllGather A is issued mid-layer so it overlaps the back half of the tile
    loop; next layer's lo-gathers depend only on it, hi-gathers on AllGather B.
"""

import sys

if "/opt/trn_rl_repo" not in sys.path:
    sys.path.insert(0, "/opt/trn_rl_repo")

from contextlib import ExitStack

import numpy as np
import ml_dtypes

N_NODES = 50000
F = 128
OUT_F = 64
NCORES = 8
NLOC = 6272          # nodes per core
NTILES = 49          # 6272 / 128
NPAD = NCORES * NLOC  # 50176
P = 128
AT = 24              # tiles per core in table A
BT = NTILES - AT     # 25 tiles in table B
RA = AT * P          # 3072 rows per core in A
RB = BT * P          # 3200 rows per core in B
NA = NCORES * RA     # 24576
NB = NCORES * RB     # 25600
COLL_A_AFTER = 27    # emit AllGather-A after this tile's gathers are queued
NGRP = NCORES * NTILES * 2  # (tile, lo/hi) buckets

_prog_cache = {}


def _wrap_idx_flat(a):
    """[n] idx stream (n % 16 == 0) -> wrapped [128, n/16] int16."""
    n = a.shape[0]
    w = a.reshape(n // 16, 16).T            # [16, n/16]
    w = np.tile(w, (8, 1))                  # [128, n/16]
    return np.ascontiguousarray(w.astype(np.int16))


def _preprocess(src, dst):
    """Bucket edges by (core,tile,lo/hi), pair-dedup same-src edges, build
    variable-length gather streams (exact cross-core max, x16)."""
    bf = ml_dtypes.bfloat16
    src = src.astype(np.int64)
    dst = dst.astype(np.int64)
    E = src.shape[0]

    gtile = dst // P
    dstloc = dst % P
    c_src = src // NLOC
    r_src = src % NLOC
    lo = r_src < RA
    tabidx = np.where(lo, c_src * RA + r_src, c_src * RB + (r_src - RA))
    bucket = gtile * 2 + (~lo).astype(np.int64)     # 0..783

    # occurrence index within (bucket, tabidx)
    ord1 = np.lexsort((tabidx, bucket))
    b_s = bucket[ord1]
    s_s = tabidx[ord1]
    d_s = dstloc[ord1]
    new = np.ones(E, bool)
    new[1:] = (b_s[1:] != b_s[:-1]) | (s_s[1:] != s_s[:-1])
    runid = np.cumsum(new) - 1
    runstart = np.flatnonzero(new)
    occ = np.arange(E) - runstart[runid]

    # rows: one per (bucket, src, occ//2). slot2 (occ%2==1) rides as 2nd dst.
    is_row = (occ % 2) == 0
    nxt_same = np.zeros(E, bool)
    nxt_same[:-1] = ~new[1:]
    has2_stream = is_row & nxt_same
    d2_stream = np.empty(E, np.int64)
    d2_stream[:-1] = d_s[1:]
    d2_stream[-1] = P
    rows_b = b_s[is_row]
    rows_s = s_s[is_row]
    rows_d1 = d_s[is_row]
    rows_h2 = has2_stream[is_row]
    rows_d2 = np.where(rows_h2, d2_stream[is_row], P)

    # group rows per bucket, multi-dst rows first
    ord2 = np.lexsort((~rows_h2, rows_b))
    rb = rows_b[ord2]
    rs = rows_s[ord2]
    rd1 = rows_d1[ord2]
    rd2 = rows_d2[ord2]
    rcnt = np.bincount(rb, minlength=NGRP)                 # rows per bucket
    r2cnt = np.bincount(rb[rows_h2[ord2]], minlength=NGRP)  # 2nd-dst rows
    rstart = np.zeros(NGRP + 1, np.int64)
    np.cumsum(rcnt, out=rstart[1:])
    rpos = np.arange(len(rb)) - rstart[rb]

    # per-(tile,grp) stream length: exact max over cores, x16, >= 16
    rc = rcnt.reshape(NCORES, NTILES, 2)
    r2c = r2cnt.reshape(NCORES, NTILES, 2)
    lens = rc.max(axis=0)                     # [NTILES, 2]
    lens = np.maximum((lens + 15) // 16 * 16, 16)
    nch = -(-lens // P)                       # chunks per (tile, grp)
    nx = -(-r2c.max(axis=0) // P)             # 2nd-pass chunks per (tile, grp)

    lenlo = tuple(int(x) for x in lens[:, 0])
    lenhi = tuple(int(x) for x in lens[:, 1])
    nxlo = tuple(int(x) for x in nx[:, 0])
    nxhi = tuple(int(x) for x in nx[:, 1])

    # slot arrays, concatenated variable-width per (tile, grp)
    off_idx = np.zeros((NTILES, 2), np.int64)      # idx-stream offsets
    off_ch = np.zeros((NTILES, 2), np.int64)       # chunk offsets (main)
    off_x = np.zeros((NTILES, 2), np.int64)        # chunk offsets (pass 2)
    acc_i = acc_c = acc_x = 0
    for t in range(NTILES):
        for g in range(2):
            off_idx[t, g] = acc_i
            off_ch[t, g] = acc_c
            off_x[t, g] = acc_x
            acc_i += lens[t, g]
            acc_c += nch[t, g]
            acc_x += nx[t, g]
    TOTI, TOTCH, TOTX = acc_i, acc_c, max(acc_x, 1)

    idx_slot = np.zeros((NCORES, TOTI), np.int64)
    oh_slot = np.full((NCORES, TOTCH * P), P, np.int64)
    oh2_slot = np.full((NCORES, TOTX * P), P, np.int64)

    core_of = rb // (NTILES * 2)
    t_of = (rb // 2) % NTILES
    g_of = rb % 2
    col_i = off_idx[t_of, g_of] + rpos
    idx_slot[core_of, col_i] = rs
    col_o = off_ch[t_of, g_of] * P + rpos
    oh_slot[core_of, col_o] = rd1
    m2 = (rd2 != P) & (rpos < nx[t_of, g_of] * P)
    col_x = off_x[t_of, g_of] * P + rpos
    oh2_slot[core_of[m2], col_x[m2]] = rd2[m2]

    deg = np.bincount(dst, minlength=NPAD).astype(np.float32)
    inv_deg = 1.0 / np.maximum(deg, 1.0)

    per_core = []
    for c in range(NCORES):
        idxs = _wrap_idx_flat(idx_slot[c])                     # [128, TOTI/16]
        dstlocb = np.ascontiguousarray(
            oh_slot[c].reshape(TOTCH, P).T).astype(bf)         # [128, TOTCH]
        dstlocb2 = np.ascontiguousarray(
            oh2_slot[c].reshape(TOTX, P).T).astype(bf)         # [128, TOTX]
        invd = inv_deg[c * NLOC:(c + 1) * NLOC].reshape(NTILES, P).T.copy()
        per_core.append(dict(idxs=idxs, dstlocb=dstlocb, dstlocb2=dstlocb2,
                             invdeg=invd))
    shape_key = (lenlo, lenhi, nxlo, nxhi)
    return per_core, shape_key


def _build_program(shape_key):
    import concourse.bacc as bacc
    import concourse.mybir as mybir
    import concourse.tile as tile

    lenlo, lenhi, nxlo, nxhi = shape_key
    nch_lo = [-(-v // P) for v in lenlo]
    nch_hi = [-(-v // P) for v in lenhi]
    NCHMAX = max(a + b for a, b in zip(nch_lo, nch_hi))
    NXMAX = max(a + b for a, b in zip(nxlo, nxhi))
    TOTI = sum(lenlo) + sum(lenhi)
    TOTCH = sum(nch_lo) + sum(nch_hi)
    TOTX = max(sum(nxlo) + sum(nxhi), 1)
    # offsets in emission order (t, lo), (t, hi)
    off_i, off_c, off_x = {}, {}, {}
    ai = ac = ax = 0
    for t in range(NTILES):
        for g, (ln, nc_, nx_) in enumerate((
                (lenlo[t], nch_lo[t], nxlo[t]), (lenhi[t], nch_hi[t], nxhi[t]))):
            off_i[t, g] = ai
            off_c[t, g] = ac
            off_x[t, g] = ax
            ai += ln
            ac += nc_
            ax += nx_

    dt = mybir.dt
    nc = bacc.Bacc("TRN2", target_bir_lowering=False, debug=False,
                   num_devices=NCORES, dynamic_dma_scratch_size=49152,
                   num_swdge_queues=4)

    htabA0 = nc.dram_tensor("htabA0", [NA, F], dt.bfloat16, kind="ExternalInput")
    htabB0 = nc.dram_tensor("htabB0", [NB, F], dt.bfloat16, kind="ExternalInput")
    hselfT0 = nc.dram_tensor("hselfT0", [F, NTILES, P], dt.bfloat16, kind="ExternalInput")
    idxs_d = nc.dram_tensor("idxs", [P, TOTI // 16], dt.int16, kind="ExternalInput")
    dstlocb = nc.dram_tensor("dstlocb", [P, TOTCH], dt.bfloat16, kind="ExternalInput")
    dstlocb2 = nc.dram_tensor("dstlocb2", [P, TOTX], dt.bfloat16, kind="ExternalInput")
    iotam = nc.dram_tensor("iotam", [P, P], dt.bfloat16, kind="ExternalInput")
    invdeg = nc.dram_tensor("invdeg", [P, NTILES], dt.float32, kind="ExternalInput")
    ident = nc.dram_tensor("ident", [P, P], dt.bfloat16, kind="ExternalInput")
    ws = [nc.dram_tensor(f"ws{l}", [F, F if l < 2 else OUT_F], dt.bfloat16,
                         kind="ExternalInput") for l in range(3)]
    wn = [nc.dram_tensor(f"wn{l}", [F, F if l < 2 else OUT_F], dt.bfloat16,
                         kind="ExternalInput") for l in range(3)]
    bs = [nc.dram_tensor(f"b{l}", [F if l < 2 else OUT_F, 1], dt.float32,
                         kind="ExternalInput") for l in range(3)]
    outd = nc.dram_tensor("outT", [OUT_F, NTILES, P], dt.float32, kind="ExternalOutput")

    tabsA = [htabA0,
             nc.dram_tensor("htabA1", [NA, F], dt.bfloat16, addr_space="Shared"),
             nc.dram_tensor("htabA2", [NA, F], dt.bfloat16, addr_space="Shared")]
    tabsB = [htabB0,
             nc.dram_tensor("htabB1", [NB, F], dt.bfloat16, addr_space="Shared"),
             nc.dram_tensor("htabB2", [NB, F], dt.bfloat16, addr_space="Shared")]
    blkA = [nc.dram_tensor(f"blkA{l}", [RA, F], dt.bfloat16) for l in range(2)]
    blkB = [nc.dram_tensor(f"blkB{l}", [RB, F], dt.bfloat16) for l in range(2)]

    with tile.TileContext(nc) as tc, ExitStack() as ctx:
        const = ctx.enter_context(tc.tile_pool(name="const", bufs=1))
        stpool = ctx.enter_context(tc.tile_pool(name="stage", bufs=1))
        msgp = ctx.enter_context(tc.tile_pool(name="msg", bufs=8))
        ohp = ctx.enter_context(tc.tile_pool(name="oh", bufs=3))
        sbw = ctx.enter_context(tc.tile_pool(name="work", bufs=4))
        psA = ctx.enter_context(tc.tile_pool(name="psA", bufs=2, space="PSUM"))
        psT = ctx.enter_context(tc.tile_pool(name="psT", bufs=2, space="PSUM"))
        psO = ctx.enter_context(tc.tile_pool(name="psO", bufs=2, space="PSUM"))

        def load(t, d):
            nc.sync.dma_start(t[:], d[:])
            return t

        idxs_sb = load(const.tile([P, TOTI // 16], dt.int16, name="idxs_sb"), idxs_d)
        dstlocb_sb = load(const.tile([P, TOTCH], dt.bfloat16, name="dstlocb_sb"), dstlocb)
        dstlocb2_sb = load(const.tile([P, TOTX], dt.bfloat16, name="dstlocb2_sb"), dstlocb2)
        iotam_sb = load(const.tile([P, P], dt.bfloat16, name="iotam_sb"), iotam)
        invdeg_sb = load(const.tile([P, NTILES], dt.float32, name="invdeg_sb"), invdeg)
        ident_sb = load(const.tile([P, P], dt.bfloat16, name="ident_sb"), ident)
        ws_sb = [load(const.tile([F, F if l < 2 else OUT_F], dt.bfloat16, name=f"ws_sb{l}"), ws[l])
                 for l in range(3)]
        wn_sb = [load(const.tile([F, F if l < 2 else OUT_F], dt.bfloat16, name=f"wn_sb{l}"), wn[l])
                 for l in range(3)]
        bs_sb = [load(const.tile([F if l < 2 else OUT_F, 1], dt.float32, name=f"bs_sb{l}"), bs[l])
                 for l in range(3)]

        stageA = load(stpool.tile([F, NTILES, P], dt.bfloat16, name="stageA", tag="stA"), hselfT0)
        stageB = stpool.tile([F, NTILES, P], dt.bfloat16, tag="stB")
        outstage = stpool.tile([OUT_F, NTILES, P], dt.float32, tag="stO")

        # prime msg buffers: pad slots in partially-gathered chunks are never
        # written and must stay finite (0 * 0 = 0 in the matmul).
        for _ in range(8):
            m = msgp.tile([P, NCHMAX, F], dt.bfloat16, tag="msg")
            nc.vector.memset(m[:], 0.0)

        stage_prev, stage_next = stageA, stageB
        for l in range(3):
            tabA, tabB = tabsA[l], tabsB[l]
            OUTL = F if l < 2 else OUT_F
            for t in range(NTILES):
                nlo, nhi = nch_lo[t], nch_hi[t]
                ncht = nlo + nhi
                msg = msgp.tile([P, NCHMAX, F], dt.bfloat16, tag="msg")
                nc.gpsimd.dma_gather(
                    msg[:, 0:nlo, :], tabA[:, :],
                    idxs_sb[:, off_i[t, 0] // 16:(off_i[t, 0] + lenlo[t]) // 16],
                    num_idxs=lenlo[t], num_idxs_reg=lenlo[t], elem_size=F,
                    single_packet=False, queue_num=(2 * t) % 4)
                nc.gpsimd.dma_gather(
                    msg[:, nlo:ncht, :], tabB[:, :],
                    idxs_sb[:, off_i[t, 1] // 16:(off_i[t, 1] + lenhi[t]) // 16],
                    num_idxs=lenhi[t], num_idxs_reg=lenhi[t], elem_size=F,
                    single_packet=False, queue_num=(2 * t + 1) % 4)

                if l < 2 and t == COLL_A_AFTER:
                    nc.gpsimd.collective_compute(
                        "AllGather", mybir.AluOpType.bypass,
                        replica_groups=[list(range(NCORES))],
                        ins=[blkA[l][:]], outs=[tabsA[l + 1][:]])

                oh = ohp.tile([P, NCHMAX + NXMAX, P], dt.bfloat16, tag="oh")
                nc.vector.tensor_tensor(
                    oh[:, 0:ncht, :],
                    iotam_sb[:].unsqueeze(1).to_broadcast([P, ncht, P]),
                    dstlocb_sb[:, off_c[t, 0]:off_c[t, 0] + ncht]
                    .unsqueeze(2).to_broadcast([P, ncht, P]),
                    mybir.AluOpType.is_equal)
                nxt = nxlo[t] + nxhi[t]
                if nxt:
                    nc.vector.tensor_tensor(
                        oh[:, ncht:ncht + nxt, :],
                        iotam_sb[:].unsqueeze(1).to_broadcast([P, nxt, P]),
                        dstlocb2_sb[:, off_x[t, 0]:off_x[t, 0] + nxt]
                        .unsqueeze(2).to_broadcast([P, nxt, P]),
                        mybir.AluOpType.is_equal)

                # (one-hot chunk, msg chunk) pairs: main pass + 2nd-dst pass
                pairs = [(k, k) for k in range(ncht)]
                pairs += [(ncht + j, j) for j in range(nxlo[t])]
                pairs += [(ncht + nxlo[t] + j, nlo + j) for j in range(nxhi[t])]
                agg = psA.tile([P, F], dt.float32, tag="agg")
                for i, (ko, km) in enumerate(pairs):
                    nc.tensor.matmul(agg[:], oh[:, ko, :], msg[:, km, :],
                                     start=(i == 0), stop=(i == len(pairs) - 1))
                hn = sbw.tile([P, F], dt.bfloat16, tag="hn")
                nc.vector.tensor_scalar_mul(hn[:], agg[:], invdeg_sb[:, t:t + 1])

                hnT_ps = psT.tile([F, P], dt.bfloat16, tag="hnT")
                nc.tensor.transpose(hnT_ps[:], hn[:], ident_sb[:])
                hnT = sbw.tile([F, P], dt.bfloat16, tag="hnTs")
                nc.vector.tensor_copy(hnT[:], hnT_ps[:])

                outp = psO.tile([OUTL, P], dt.float32, tag="outp")
                nc.tensor.matmul(outp[:], ws_sb[l][:], stage_prev[:, t, :],
                                 start=True, stop=False)
                nc.tensor.matmul(outp[:], wn_sb[l][:], hnT[:],
                                 start=False, stop=True)

                if l < 2:
                    nc.scalar.activation(stage_next[:, t, :], outp[:],
                                         mybir.ActivationFunctionType.Relu,
                                         bias=bs_sb[l][:], scale=1.0)
                    oT_ps = psT.tile([P, F], dt.bfloat16, tag="oT")
                    nc.tensor.transpose(oT_ps[:], stage_next[:, t, :], ident_sb[:])
                    rowm = sbw.tile([P, F], dt.bfloat16, tag="rowm")
                    nc.vector.tensor_copy(rowm[:], oT_ps[:])
                    if t < AT:
                        nc.sync.dma_start(blkA[l][t * P:(t + 1) * P, :], rowm[:])
                    else:
                        nc.sync.dma_start(
                            blkB[l][(t - AT) * P:(t - AT + 1) * P, :], rowm[:])
                else:
                    nc.vector.tensor_scalar_add(outstage[:, t, :], outp[:],
                                                bs_sb[2][:])

            if l < 2:
                nc.gpsimd.collective_compute(
                    "AllGather", mybir.AluOpType.bypass,
                    replica_groups=[list(range(NCORES))],
                    ins=[blkB[l][:]], outs=[tabsB[l + 1][:]])
                stage_prev, stage_next = stage_next, stage_prev

        nc.sync.dma_start(outd[:], outstage[:])

    nc.compile()
    return nc


def kernel(features, src, dst, W0, b0, W1, b1, W2, b2):
    features = np.asarray(features, np.float32)
    src = np.asarray(src)
    dst = np.asarray(dst)

    per_core, shape_key = _preprocess(src, dst)

    if shape_key not in _prog_cache:
        _prog_cache[shape_key] = _build_program(shape_key)
    nc = _prog_cache[shape_key]

    bf = ml_dtypes.bfloat16
    feat_pad = np.zeros((NPAD, F), np.float32)
    feat_pad[:N_NODES] = features
    fp = feat_pad.reshape(NCORES, NLOC, F)
    htabA0 = np.ascontiguousarray(fp[:, :RA].reshape(NA, F)).astype(bf)
    htabB0 = np.ascontiguousarray(fp[:, RA:].reshape(NB, F)).astype(bf)
    ident = np.eye(P, dtype=bf)
    iotam = np.tile(np.arange(P, dtype=np.float32), (P, 1)).astype(bf)
    Wl = [np.asarray(w, np.float32) for w in (W0, W1, W2)]
    bl = [np.asarray(b, np.float32).reshape(-1, 1) for b in (b0, b1, b2)]

    common = dict(htabA0=htabA0, htabB0=htabB0, iotam=iotam, ident=ident)
    for l in range(3):
        common[f"ws{l}"] = Wl[l][:F].astype(bf)
        common[f"wn{l}"] = Wl[l][F:].astype(bf)
        common[f"b{l}"] = bl[l]

    in_maps = []
    for c in range(NCORES):
        m = dict(common)
        m.update(per_core[c])
        hs = feat_pad[c * NLOC:(c + 1) * NLOC].reshape(NTILES, P, F)
        m["hselfT0"] = np.ascontiguousarray(hs.transpose(2, 0, 1)).astype(bf)
        in_maps.append(m)

    from concourse.bass_utils import run_bass_kernel_spmd
    res = run_bass_kernel_spmd(nc, in_maps, core_ids=list(range(NCORES)))
    global last_result
    last_result = res
    out = np.empty((NPAD, OUT_F), np.float32)
    for c in range(NCORES):
        oT = res.results[c]["outT"]  # [OUT_F, NTILES, P]
        out[c * NLOC:(c + 1) * NLOC] = np.asarray(oT).transpose(1, 2, 0).reshape(NLOC, OUT_F)
    return np.ascontiguousarray(out[:N_NODES]).astype(np.float32)


last_result = None
